# revision 2
# baseline (speedup 1.0000x reference)
"""Channel-wise (XCA / XCiT-style) self-attention Trainium2 kernel, v2.

Key algebraic reduction vs v1: the attention logits only need channel-channel
grams over the N=16384 spatial axis:
    qT k = Wq~^T (x~ x~^T) Wq~,   diag(q^T q), diag(k^T k) likewise,
where x~ = [x; ones] (193 x N) and W~ = [W; b] (193 x .) make the qkv bias
exact. So pass 1 collapses to the 193x193 gram  M~ = x~ x~^T  (one fp16
matmul pass over x^T tiles), and the whole q/k/softmax pipeline becomes
193-sized matrix algebra ("tiny phase"):
    G2 = M~ W~qk;  S^T_g = G2k_g^T W~q_g;  kgram/qgram for the norm diags;
    softmax -> A_g;  Wf = W~v (A W_proj) (+ b_proj via a K=1 matmul).
Pass 2 (unchanged in spirit): out[n,c'] = x~^T Wf in (N,C) layout,
one [128n,192] tile per matmul pair; host undoes the transpose.

Per-core FLOPs drop from ~5.4 GF to ~2.45 GF; there is no per-subtile
vector work at all in pass 1. DMA: x^T tiles (packed, 6.36 MB), x~ in
(C,N) layout (6.33 MB), out 6.29 MB.

Sharding: data-parallel over batch B=8, one batch per NeuronCore.
"""

import numpy as np

B, C, HH, WW = 8, 192, 128, 128
N = HH * WW
CA = C + 1  # augmented channels (ones row for bias folding)
NHEADS, DH, G, GC = 8, 24, 2, 96
NT = N // 128  # 128-row n-tiles
NEG_BIG = -1.0e30

_BUILT = {}


def _patch_tile_drain():
    """The final TileContext drain carries one sem wait per live processor;
    this container's walrus codegen only accepts a single sync wait on the
    CTRL Drain. Split the waits across a chain of drains (1 wait each)."""
    import bass_rust
    import concourse.tile as tile
    from concourse.vector_clock import ScopedClock

    if getattr(tile.TileContext, "_drain_split_patch", False):
        return

    def _split_drain_and_barrier(self, tick_clock, wait_clock):
        nc = self.nc
        drain_bi = nc.sync.drain()
        wait_clock.add_sem_waits(
            drain_bi.ins, ScopedClock({None: tick_clock.global_clock})
        )
        inst = drain_bi.ins
        si = inst.sync_info
        if si is not None:
            waits = list(si.on_wait or [])
            ups = list(si.on_update or [])
            if len(waits) > 1:
                inst.sync_info = bass_rust.SyncInfo(on_wait=[waits[0]], on_update=[])
                for i, w in enumerate(waits[1:]):
                    extra = nc.sync.drain()
                    last = i == len(waits) - 2
                    extra.ins.sync_info = bass_rust.SyncInfo(
                        on_wait=[w], on_update=ups if last else []
                    )
        nc.all_engine_barrier()
        assert self.sems is not None
        popped = nc._tile_sem_poison_stack.pop()
        assert popped is self._sem_poison
        nc.clear_and_free_semaphores(list(self.sems.allocated().values()))
        nc.all_engine_barrier()

    tile.TileContext._drain_and_barrier = _split_drain_and_barrier
    tile.TileContext._drain_split_patch = True


def _split_excess_waits(nc, max_waits=1):
    """Move excess sync waits onto NoOp carriers inserted just before the
    instruction on the same engine (streams process waits in issue order)."""
    import bass_rust
    from concourse import mybir

    for f in nc.m.functions:
        for bb in f.blocks:
            insts = bb.instructions
            if not any(
                getattr(i, "sync_info", None) is not None
                and i.sync_info.on_wait
                and len(list(i.sync_info.on_wait)) > max_waits
                for i in insts
            ):
                continue
            newlist = []
            for inst in insts:
                si = getattr(inst, "sync_info", None)
                if si is not None and si.on_wait:
                    waits = list(si.on_wait)
                    if len(waits) > max_waits:
                        keep = waits[-max_waits:]
                        for wi, w in enumerate(waits[: -max_waits]):
                            es = mybir.InstNoOp(
                                name=f"{inst.name}-xw{wi}", ins=[], outs=[]
                            )
                            es.engine = inst.engine
                            es.sync_info = bass_rust.SyncInfo(
                                on_wait=[w], on_update=[]
                            )
                            newlist.append(es)
                        inst.sync_info = bass_rust.SyncInfo(
                            on_wait=keep, on_update=list(si.on_update or [])
                        )
                newlist.append(inst)
            bb.instructions = newlist


try:  # fp8 xt stream needs ml_dtypes on the host for packing
    from ml_dtypes import float8_e4m3fn as _XT_F8_NP
    XT_FP8 = True
except ImportError:
    _XT_F8_NP = None
    XT_FP8 = False
P2CN = False


def _build(n_total=N, split=True, repeat=1, stages=("g", "t", "p2"), xt_fp8=None,
           p2cn=None, warm=False):
    """Build the single-core Bass program. Returns nc."""
    import contextlib as _ctxlib

    import concourse.bass as bass
    import concourse.tile as tile
    from concourse import mybir

    _patch_tile_drain()

    if xt_fp8 is None:
        xt_fp8 = XT_FP8
    if p2cn is None:
        p2cn = P2CN
    f32 = mybir.dt.float32
    f16 = mybir.dt.float16
    xdt = mybir.dt.float8e4 if xt_fp8 else f16
    AFT = mybir.ActivationFunctionType
    ALU = mybir.AluOpType
    AX = mybir.AxisListType

    nt = n_total // 128
    TCH = 32 if xt_fp8 else 16  # n-tiles per xt DMA chunk
    NCHX = nt // TCH
    OTCH = 16  # n-tiles per out chunk ((N,C) path)
    XC = n_total // 4  # x0/x1 DMA chunk width (4 chunks)

    nc = bass.Bass("TRN2", target_bir_lowering=False, debug=False)

    xt = nc.dram_tensor("xt", [128, nt * CA], xdt, kind="ExternalInput").ap()
    x0 = nc.dram_tensor("x0", [128, n_total], f16, kind="ExternalInput").ap()
    x1 = nc.dram_tensor("x1", [65, n_total], f16, kind="ExternalInput").ap()
    wqk0 = nc.dram_tensor("wqk0", [128, 2 * C], f16, kind="ExternalInput").ap()
    wqk1 = nc.dram_tensor("wqk1", [65, 2 * C], f16, kind="ExternalInput").ap()
    wvt = nc.dram_tensor("wvt", [GC, G, CA], f16, kind="ExternalInput").ap()
    wp0 = nc.dram_tensor("wp0", [GC, C], f16, kind="ExternalInput").ap()
    wp1 = nc.dram_tensor("wp1", [GC, C], f16, kind="ExternalInput").ap()
    ind65 = nc.dram_tensor("ind65", [1, 65], f16, kind="ExternalInput").ap()
    bp_row = nc.dram_tensor("bp_row", [1, C], f16, kind="ExternalInput").ap()
    tmp96 = nc.dram_tensor("tmp96", [GC, G], f32, kind="ExternalInput").ap()
    eye96 = nc.dram_tensor("eye96", [GC, GC], f32, kind="ExternalInput").ap()
    eye128 = nc.dram_tensor("eye128", [128, 128], f16, kind="ExternalInput").ap()
    eye2 = nc.dram_tensor("eye2", [GC, 2 * GC], f32, kind="ExternalInput").ap()
    bmask2 = nc.dram_tensor("bmask2", [GC, 2 * GC], f32, kind="ExternalInput").ap()
    if p2cn:
        out = nc.dram_tensor("out", [C, n_total], f16, kind="ExternalOutput").ap()
    else:
        out = nc.dram_tensor("out", [128, nt * C], f16, kind="ExternalOutput").ap()

    with tile.TileContext(nc) as tc:
        with (
            tc.tile_pool(name="const", bufs=1) as const,
            tc.tile_pool(name="xp", bufs=3) as xp,
            tc.tile_pool(name="vres", bufs=1) as vres,
            tc.tile_pool(name="small", bufs=1) as small,
            tc.tile_pool(name="op", bufs=3) as op,
            tc.tile_pool(name="psS", bufs=1, space="PSUM") as psS,
            tc.tile_pool(name="psA", bufs=2, space="PSUM") as psA,
            tc.tile_pool(name="psP", bufs=3, space="PSUM") as psP,
        ):
            # --- constants into SBUF ---
            wqk0_sb = const.tile([128, 2 * C], f16)
            nc.sync.dma_start(wqk0_sb[:], wqk0)
            wqk1_sb = const.tile([65, 2 * C], f16)
            nc.sync.dma_start(wqk1_sb[:], wqk1)
            eye128_sb = const.tile([128, 128], f16)
            nc.sync.dma_start(eye128_sb[:], eye128)
            wvt_sb = const.tile([GC, G, CA], f16, name="wvt_sb")
            nc.scalar.dma_start(wvt_sb[:], wvt)
            wp0_sb = const.tile([GC, C], f16)
            nc.scalar.dma_start(wp0_sb[:], wp0)
            wp1_sb = const.tile([GC, C], f16)
            nc.scalar.dma_start(wp1_sb[:], wp1)
            ind65_sb = const.tile([1, 65], f16)
            nc.scalar.dma_start(ind65_sb[:], ind65)
            bp_row_sb = const.tile([1, C], f16)
            nc.scalar.dma_start(bp_row_sb[:], bp_row)
            tmp96_sb = const.tile([GC, G], f32)
            nc.scalar.dma_start(tmp96_sb[:], tmp96)
            eye_sb = const.tile([GC, GC], f32)
            nc.scalar.dma_start(eye_sb[:], eye96)
            eye2_sb = const.tile([GC, 2, GC], f32)
            nc.scalar.dma_start(eye2_sb[:], eye2)
            bmask2_sb = const.tile([GC, 2, GC], f32)
            nc.scalar.dma_start(bmask2_sb[:], bmask2)

            # --- persistent tiles ---
            x0_sb = vres.tile([128, n_total], f16, tag="x0r", name="x0r")
            x1_sb = vres.tile([65, n_total], f16, tag="x1r", name="x1r")
            # PSUM is bank-granular (8 x 2KB/partition): pack the persistent
            # accumulators into 3 banks
            gq = psS.tile([128, CA + 257], f32, tag="GAB", name="GAB")
            psGA = gq[:, 0:CA]
            qdt = gq[0:GC, CA : CA + 192].rearrange("p (g i) -> p g i", g=G)
            psGB = gq[0:65, CA + 192 : CA + 257]
            sgk = psS.tile([GC, 4, GC], f32, tag="SGK", name="SGK")
            sgS = [sgk[:, 0, :], sgk[:, 1, :]]  # S^T per group
            sgK = [sgk[:, 2, :], sgk[:, 3, :]]  # k-gram per group
            qdv = [qdt[:, g, :] for g in range(G)]

            _rep_cm = (
                tc.For_i(0, repeat, 1, hint_engines=tuple(nc.engines.keys()))
                if repeat > 1
                else _ctxlib.nullcontext()
            )
            with _rep_cm:
                # ---------------- phase 1: gram M~ = x~ x~^T ----------------
                for ch in range(NCHX):
                    eng = nc.sync if ch % 2 == 0 else nc.scalar
                    xt_sb = xp.tile([128, TCH * CA], xdt, tag="xt")
                    eng.dma_start(
                        xt_sb[:], xt[:, ch * TCH * CA : (ch + 1) * TCH * CA]
                    )
                    if ch == NCHX - 1:
                        # x~ resident loads queue behind all xt chunks: the gram
                        # stream gets full DMA bandwidth, x~ lands during the
                        # tiny phase / early pass 2
                        for i in range(n_total // XC):
                            cs = slice(i * XC, (i + 1) * XC)
                            e0 = nc.sync if i % 2 == 0 else nc.scalar
                            e1 = nc.scalar if i % 2 == 0 else nc.sync
                            e0.dma_start(x0_sb[:, cs], x0[:, cs])
                            e1.dma_start(x1_sb[:, cs], x1[:, cs])
                    for tl in range(TCH if "g" in stages else 0):
                        t = ch * TCH + tl
                        xtt = xt_sb[:, tl * CA : (tl + 1) * CA]
                        first = t == 0
                        last = t == nt - 1
                        nc.tensor.matmul(
                            psGA[:],
                            xtt[:, 0:128],
                            xtt[:],
                            start=first,
                            stop=last,
                            skip_group_check=True,
                        )
                        nc.tensor.matmul(
                            psGB[:],
                            xtt[:, 128:CA],
                            xtt[:, 128:CA],
                            start=first,
                            stop=last,
                            skip_group_check=True,
                        )
                        if warm and tl % 2 == 0:
                            # phase 1 is DMA-bound; junk matmuls keep the PE
                            # HAM clock-gate at 8/8 (2.4 GHz) into phase 3
                            jk = psP.tile([GC, 512], f32, tag="P")
                            nc.tensor.matmul(
                                jk[:, 0 : 2 * C],
                                eye128_sb[:, 0:GC],
                                wqk0_sb[:],
                                start=True,
                                stop=True,
                                skip_group_check=True,
                            )

                # ---------------- tiny phase ----------------
                wt_sb = None
                if "t" not in stages and "p2" in stages:
                    wf0_sb = small.tile([128, C], f16, tag="wf0", name="wf0d")
                    nc.vector.memset(wf0_sb[:], 0.001)
                    wf1_sb = small.tile([65, C], f16, tag="wf1", name="wf1d")
                    nc.vector.memset(wf1_sb[:], 0.001)
                if "t" in stages:
                    # M~ into SBUF fp16 (m0 = rows 0:128, m1 = rows 128:193)
                    m0_sb = small.tile([128, CA], f16, tag="m0")
                    nc.scalar.activation(out=m0_sb[:], in_=psGA[:], func=AFT.Identity)
                    m1_sb = small.tile([65, CA], f16, tag="m1")
                    ps_tr0 = psA.tile([65, 128], f16, tag="A")
                    nc.tensor.transpose(ps_tr0[:], m0_sb[:, 128:CA], eye128_sb[:])
                    nc.vector.tensor_copy(m1_sb[:, 0:128], ps_tr0[:])
                    nc.vector.tensor_copy(m1_sb[:, 128:CA], psGB[:])

                    # G2 = M~ @ W~qk   (contraction over partitions, M~ symmetric)
                    ps_g2a = psA.tile([128, 2 * C], f32, tag="A")
                    nc.tensor.matmul(
                        ps_g2a[:], m0_sb[:, 0:128], wqk0_sb[:], start=True, stop=False
                    )
                    nc.tensor.matmul(
                        ps_g2a[:], m1_sb[:, 0:128], wqk1_sb[:], start=False, stop=True
                    )
                    g2a_sb = small.tile([128, 2 * C], f16, tag="g2a")
                    nc.scalar.activation(out=g2a_sb[:], in_=ps_g2a[:], func=AFT.Identity)
                    ps_g2b = psA.tile([65, 2 * C], f32, tag="A")
                    nc.tensor.matmul(
                        ps_g2b[:], m0_sb[:, 128:CA], wqk0_sb[:], start=True, stop=False
                    )
                    nc.tensor.matmul(
                        ps_g2b[:], m1_sb[:, 128:CA], wqk1_sb[:], start=False, stop=True
                    )
                    g2b_sb = small.tile([65, 2 * C], f16, tag="g2b")
                    nc.scalar.activation(out=g2b_sb[:], in_=ps_g2b[:], func=AFT.Identity)

                    # per-group small grams: S^T, kgram, qgram
                    for g in range(G):
                        qc = slice(g * GC, (g + 1) * GC)
                        kc = slice(2 * GC + g * GC, 2 * GC + (g + 1) * GC)
                        for dst, lc, rc in (
                            (sgS[g], kc, qc),
                            (sgK[g], kc, kc),
                            (qdv[g], qc, qc),
                        ):
                            nc.tensor.matmul(
                                dst,
                                g2a_sb[:, lc],
                                wqk0_sb[:, rc],
                                start=True,
                                stop=False,
                                skip_group_check=True,
                            )
                            nc.tensor.matmul(
                                dst,
                                g2b_sb[:, lc],
                                wqk1_sb[:, rc],
                                start=False,
                                stop=True,
                                skip_group_check=True,
                            )

                    # batched softmax across both groups (short serial path)
                    ss4 = small.tile([GC, 4], f32, tag="ss4")
                    trk = small.tile([GC, 2, GC], f32, tag="trk")
                    nc.vector.tensor_mul(out=trk[:], in0=sgk[:, 2:4, :], in1=eye2_sb[:])
                    nc.vector.reduce_sum(out=ss4[:, 0:2], in_=trk[:], axis=AX.X)
                    trq = small.tile([GC, 2, GC], f32, tag="trq")
                    nc.vector.tensor_mul(out=trq[:], in0=qdt[:], in1=eye2_sb[:])
                    nc.vector.reduce_sum(out=ss4[:, 2:4], in_=trq[:], axis=AX.X)
                    # r = 1/max(sqrt(ss), eps); rq gets temperature folded in
                    nc.vector.tensor_scalar_max(out=ss4[:], in0=ss4[:], scalar1=1e-24)
                    nc.scalar.sqrt(ss4[:], ss4[:])
                    nc.vector.reciprocal(ss4[:], ss4[:])
                    nc.vector.tensor_tensor(
                        out=ss4[:, 2:4], in0=ss4[:, 2:4], in1=tmp96_sb[:], op=ALU.mult
                    )
                    stw = small.tile([GC, 2, GC], f32, tag="stw")
                    for g in range(G):
                        nc.vector.tensor_scalar_mul(
                            out=stw[:, g, :], in0=sgS[g], scalar1=ss4[:, g, None]
                        )
                    ps_trp = psA.tile([GC, 2, GC], f32, tag="A")
                    for g in range(G):
                        nc.tensor.transpose(ps_trp[:, g, :], stw[:, g, :], eye_sb[:])
                    s_sb = small.tile([GC, 2, GC], f32, tag="s")
                    for g in range(G):
                        nc.vector.tensor_scalar_mul(
                            out=s_sb[:, g, :],
                            in0=ps_trp[:, g, :],
                            scalar1=ss4[:, 2 + g, None],
                        )
                    nc.vector.tensor_tensor(
                        out=s_sb[:], in0=s_sb[:], in1=bmask2_sb[:], op=ALU.add
                    )
                    # |logits| <= |temperature| (L2-normalized), so exp without
                    # max-subtraction is safe in fp32; mask -1e30 underflows to 0
                    e_sb = small.tile([GC, 2, GC], f32, tag="e")
                    nc.scalar.activation(out=e_sb[:], in_=s_sb[:], func=AFT.Exp, scale=1.0)
                    rsum = small.tile([GC, 2], f32, tag="rsum")
                    nc.vector.reduce_sum(out=rsum[:], in_=e_sb[:], axis=AX.X)
                    nc.vector.reciprocal(rsum[:], rsum[:])
                    a_sb = small.tile([GC, 2, GC], f16, tag="a")
                    for g in range(G):
                        nc.vector.tensor_scalar_mul(
                            out=a_sb[:, g, :], in0=e_sb[:, g, :], scalar1=rsum[:, g, None]
                        )
                    ps_wp = psA.tile([GC, 2, C], f32, tag="A")
                    nc.tensor.matmul(
                        ps_wp[:, 0, :], a_sb[:, 0, :], wp0_sb[:],
                        start=True, stop=True, skip_group_check=True,
                    )
                    nc.tensor.matmul(
                        ps_wp[:, 1, :], a_sb[:, 1, :], wp1_sb[:],
                        start=True, stop=True, skip_group_check=True,
                    )
                    wt_pair = small.tile([GC, 2, C], f16, tag="wtp")
                    nc.scalar.activation(out=wt_pair[:], in_=ps_wp[:], func=AFT.Identity)
                    wt_sb = [wt_pair[:, 0, :], wt_pair[:, 1, :]]

                    # Wf = W~v @ stack(wt_g)  (+ b_proj on the ones row)
                    ps_wf0 = psA.tile([128, C], f32, tag="A")
                    for g in range(G):
                        nc.tensor.matmul(
                            ps_wf0[:],
                            wvt_sb[:, g, 0:128],
                            wt_sb[g][:],
                            start=(g == 0),
                            stop=(g == G - 1),
                        )
                    wf0_sb = small.tile([128, C], f16, tag="wf0", name="wf0")
                    nc.scalar.activation(out=wf0_sb[:], in_=ps_wf0[:], func=AFT.Identity)
                    ps_wf1 = psA.tile([65, C], f32, tag="A")
                    for g in range(G):
                        nc.tensor.matmul(
                            ps_wf1[:],
                            wvt_sb[:, g, 128:CA],
                            wt_sb[g][:],
                            start=(g == 0),
                            stop=False,
                        )
                    nc.tensor.matmul(
                        ps_wf1[:], ind65_sb[:], bp_row_sb[:], start=False, stop=True
                    )
                    wf1_sb = small.tile([65, C], f16, tag="wf1", name="wf1")
                    nc.scalar.activation(out=wf1_sb[:], in_=ps_wf1[:], func=AFT.Identity)

                # ---------------- phase 3: out = x~^T Wf  (N,C layout) ----------
                if "p2" in stages and p2cn:
                    # out[c', n] = Wf^T x~ : wf column-chunks stationary,
                    # x~ moving in 512-wide streams; out stays (C,N)
                    for och in range(4):
                        cs = slice(och * XC, (och + 1) * XC)
                        ost = op.tile([GC, 2, XC], f16, tag="ost")
                        for bi in range(XC // 512):
                            bs = slice(bi * 512, (bi + 1) * 512)
                            ns = slice(och * XC + bi * 512, och * XC + (bi + 1) * 512)
                            for mc in range(2):
                                ms = slice(mc * GC, (mc + 1) * GC)
                                po = psP.tile([GC, 512], f32, tag="P")
                                nc.tensor.matmul(
                                    po[:], wf0_sb[:, ms], x0_sb[:, ns],
                                    start=True, stop=False, skip_group_check=True,
                                )
                                nc.tensor.matmul(
                                    po[:], wf1_sb[:, ms], x1_sb[:, ns],
                                    start=False, stop=True, skip_group_check=True,
                                )
                                dst = ost[:, mc, bs]
                                if mc == 0:
                                    nc.scalar.activation(
                                        out=dst, in_=po[:], func=AFT.Identity
                                    )
                                else:
                                    nc.vector.tensor_copy(dst, po[:])
                        e0 = nc.sync if och % 2 == 0 else nc.scalar
                        e1 = nc.scalar if och % 2 == 0 else nc.sync
                        e0.dma_start(out[0:GC, cs], ost[:, 0, :])
                        e1.dma_start(out[GC:C, cs], ost[:, 1, :])
                elif "p2" in stages:
                    for och in range(nt // OTCH):
                        ost = op.tile([128, OTCH * C], f16, tag="ost")
                        for p in range(OTCH // 2):
                            t0 = och * OTCH + 2 * p
                            po2 = psP.tile([128, 2, C], f32, tag="P")
                            for j in range(2):
                                ns = slice((t0 + j) * 128, (t0 + j + 1) * 128)
                                nc.tensor.matmul(
                                    po2[:, j, :], x0_sb[:, ns], wf0_sb[:],
                                    start=True, stop=False, skip_group_check=True,
                                )
                                nc.tensor.matmul(
                                    po2[:, j, :], x1_sb[:, ns], wf1_sb[:],
                                    start=False, stop=True, skip_group_check=True,
                                )
                            dst = ost[:, 2 * p * C : (2 * p + 2) * C]
                            if p % 2 == 0:
                                nc.scalar.activation(
                                    out=dst, in_=po2[:], func=AFT.Identity
                                )
                            else:
                                nc.vector.tensor_copy(dst, po2[:])
                        eng = nc.sync if och % 2 == 0 else nc.scalar
                        eng.dma_start(
                            out[:, och * OTCH * C : (och + 1) * OTCH * C], ost[:]
                        )
                elif "od" in stages:
                    pass

    if split:
        _split_excess_waits(nc)
    return nc


def _host_aux(W_qkv, b_qkv, temperature, W_proj, b_proj):
    W_qkv = np.asarray(W_qkv, dtype=np.float32)
    b_qkv = np.asarray(b_qkv, dtype=np.float32)
    temperature = np.asarray(temperature, dtype=np.float32).reshape(NHEADS)
    W_proj = np.asarray(W_proj, dtype=np.float32)
    b_proj = np.asarray(b_proj, dtype=np.float32)

    f16 = np.float16
    Wqk_aug = np.vstack([W_qkv[:, 0 : 2 * C], b_qkv[None, 0 : 2 * C]])  # (193, 384)
    Wv_aug = np.vstack([W_qkv[:, 2 * C : 3 * C], b_qkv[None, 2 * C : 3 * C]])
    ind65 = np.zeros((1, 65), np.float32)
    ind65[0, 64] = 1.0
    aux = {
        "wqk0": np.ascontiguousarray(Wqk_aug[0:128]).astype(f16),
        "wqk1": np.ascontiguousarray(Wqk_aug[128:CA]).astype(f16),
        "wvt": np.ascontiguousarray(
            Wv_aug.T.reshape(G, GC, CA).transpose(1, 0, 2)
        ).astype(f16),
        "wp0": np.ascontiguousarray(W_proj[0:GC, :]).astype(f16),
        "wp1": np.ascontiguousarray(W_proj[GC:C, :]).astype(f16),
        "ind65": ind65.astype(f16),
        "bp_row": np.ascontiguousarray(b_proj[None, :]).astype(f16),
        "tmp96": np.ascontiguousarray(
            np.stack(
                [np.repeat(temperature[4 * g : 4 * (g + 1)], DH) for g in range(G)],
                axis=1,
            )
        ),
        "eye96": np.eye(GC, dtype=np.float32),
        "eye128": np.eye(128, dtype=f16),
        "eye2": np.tile(np.eye(GC, dtype=np.float32), (1, 2)),
        "bmask2": np.tile(
            np.where(
                np.kron(np.eye(4, dtype=bool), np.ones((DH, DH), dtype=bool)),
                np.float32(0.0),
                np.float32(NEG_BIG),
            ).astype(np.float32),
            (1, 2),
        ),
    }
    return aux


def make_in_maps(inputs):
    x = np.asarray(inputs["x"], dtype=np.float32).reshape(B, C, N)
    aux = _host_aux(
        inputs["W_qkv"], inputs["b_qkv"], inputs["temperature"],
        inputs["W_proj"], inputs["b_proj"],
    )
    f16 = np.float16
    xt_np_dt = _XT_F8_NP if XT_FP8 else f16
    ones = np.ones((1, N), f16)
    in_maps = []
    for b in range(B):
        xa = np.concatenate([x[b].astype(f16), ones], axis=0)  # (193, N)
        xtp = np.ascontiguousarray(
            xa.T.reshape(NT, 128, CA).transpose(1, 0, 2).reshape(128, NT * CA)
        ).astype(xt_np_dt)
        in_maps.append(
            {
                "xt": xtp,
                "x0": np.ascontiguousarray(xa[0:128]),
                "x1": np.ascontiguousarray(xa[128:CA]),
                **aux,
            }
        )
    return in_maps


def unpack_out(res_out):
    if P2CN:
        return np.asarray(res_out).reshape(C, HH, WW)
    o = np.asarray(res_out).reshape(128, NT, C).transpose(1, 0, 2).reshape(N, C)
    return o.T.reshape(C, HH, WW)


def kernel(x, W_qkv, b_qkv, temperature, W_proj, b_proj):
    from concourse.bass_utils import run_bass_kernel_spmd

    if "nc" not in _BUILT:
        _BUILT["nc"] = _build(N)
    nc = _BUILT["nc"]

    in_maps = make_in_maps(
        {
            "x": x, "W_qkv": W_qkv, "b_qkv": b_qkv,
            "temperature": temperature, "W_proj": W_proj, "b_proj": b_proj,
        }
    )
    res = run_bass_kernel_spmd(nc, in_maps, core_ids=list(range(B)))
    out = np.stack([unpack_out(res.results[b]["out"]) for b in range(B)], axis=0)
    return out.astype(np.float32)
